# revision 6
# baseline (speedup 1.0000x reference)
"""KAN layer (identity edges) Trainium2 kernel.

output[b, o] = sum_i x[b, i]  for all o  -- row-sum broadcast to (B, 1024).

Data-parallel over 8 NeuronCores: each core gets 8192 rows of x
(65536 x 1024 f32), computes row sums on the Vector engine, broadcasts
them across the feature dim on-chip (cast to bf16, well within the 2e-2
tolerance), and DMAs the (8192, 1024) bf16 shard out; the host upcasts
back to f32.

Perf notes (HW-measured):
- bf16 store halves write traffic: 48 MiB/core total vs 64 MiB for f32
  (l2 rel err 1.7e-3, vs the 2e-2 gate).
- Load tile = [128 partitions, 4 rows, 1024] f32 and store DMAs grouped
  over two tiles [128, 8, 1024] bf16, so BOTH directions use 16 KiB
  contiguous per-partition descriptors -- this keeps ~15.9 of 16 SDMA
  engines busy (~422 GB/s, the SBUF AXI port ceiling) when the sibling
  NeuronCore leaves HBM headroom; with full sibling overlap the
  per-stack HBM split (~716/2 GB/s) binds instead.
- Loads issue on the SP HWDGE ring, stores on the ACT ring.
- The last two tiles are stored individually, and the final tile's
  reduce/cast/store chain is split in halves with the first half's cast
  on the Scalar engine, so the end-of-stream serial tail is minimized.

Layout: partition p owns 64 consecutive DRAM rows (rearrange
"(p n) d -> p n d"), so each DMA moves contiguous bytes per partition.
"""

import numpy as np

import concourse.tile as tile
from concourse import bacc, mybir
from concourse.bass_utils import run_bass_kernel_spmd

N_CORES = 8
BATCH = 65536
FEAT = 1024
ROWS = BATCH // N_CORES        # 8192 rows per core
P = 128                        # SBUF partitions
ROWS_PER_PART = ROWS // P      # 64 consecutive rows owned by each partition

R = 4                          # rows-per-partition per load tile
N_ITER = ROWS_PER_PART // R    # 16 load tiles
IN_BUFS = 5
OUT_BUFS = 3

_nc_cache = []


def _build():
    nc = bacc.Bacc()
    x = nc.declare_dram_parameter("x", [ROWS, FEAT], mybir.dt.float32, isOutput=False)
    y = nc.declare_dram_parameter("y", [ROWS, FEAT], mybir.dt.bfloat16, isOutput=True)
    xv = x[:, :].rearrange("(p n) d -> p n d", p=P)
    yv = y[:, :].rearrange("(p n) d -> p n d", p=P)

    # Pairs of load tiles share one store DMA (16 KiB bf16 descriptors);
    # the last two tiles store individually to keep the tail chain short.
    groups = [[i, i + 1] for i in range(0, N_ITER - 2, 2)] + [[N_ITER - 2], [N_ITER - 1]]

    with tile.TileContext(nc) as tc:
        with (
            tc.tile_pool(name="inp", bufs=IN_BUFS) as inp,
            tc.tile_pool(name="outp", bufs=OUT_BUFS) as outp,
            tc.tile_pool(name="sums", bufs=4) as sums_pool,
        ):
            for grp in groups:
                gr = R * len(grp)
                o = outp.tile([P, gr, FEAT], mybir.dt.bfloat16, name="o", tag="o")
                for gi, it in enumerate(grp):
                    t = inp.tile([P, R, FEAT], mybir.dt.float32, name="t", tag="t")
                    nc.sync.dma_start(
                        out=t[:, :, :], in_=xv[:, it * R : (it + 1) * R, :]
                    )

                    s = sums_pool.tile([P, R], mybir.dt.float32, name="s", tag="s")
                    last = it == N_ITER - 1
                    k = 2 if last else 1
                    step = R // k
                    for j in range(k):
                        la, lb = j * step, (j + 1) * step          # within t/s
                        a, b = gi * R + la, gi * R + lb            # within o
                        nc.vector.reduce_sum(
                            out=s[:, la:lb],
                            in_=t[:, la:lb, :],
                            axis=mybir.AxisListType.X,
                        )
                        src = s[:, la:lb].to_broadcast([P, step, FEAT])
                        if k > 1 and j == 0:
                            nc.scalar.copy(o[:, a:b, :], src)
                        else:
                            nc.vector.tensor_copy(out=o[:, a:b, :], in_=src)
                        if k > 1:
                            nc.scalar.dma_start(
                                out=yv[:, it * R + la : it * R + lb, :],
                                in_=o[:, a:b, :],
                            )
                    if k == 1 and gi == len(grp) - 1:
                        base = grp[0] * R
                        nc.scalar.dma_start(
                            out=yv[:, base : base + gr, :], in_=o[:, :, :]
                        )
    nc.finalize()
    return nc


def _get_nc():
    if not _nc_cache:
        _nc_cache.append(_build())
    return _nc_cache[0]


def kernel(x: np.ndarray) -> np.ndarray:
    nc = _get_nc()
    x = np.ascontiguousarray(np.asarray(x), dtype=np.float32)
    shards = np.split(x, N_CORES, axis=0)
    in_maps = [{"x": s} for s in shards]
    res = run_bass_kernel_spmd(nc, in_maps, list(range(N_CORES)))
    out = np.concatenate([res.results[i]["y"] for i in range(N_CORES)], axis=0)
    return out.astype(np.float32)


# revision 7
# speedup vs baseline: 1.4286x; 1.4286x over previous
"""KAN layer (identity edges) Trainium2 kernel.

output[b, o] = sum_i x[b, i]  for all o  -- row-sum broadcast to (B, 1024).

Data-parallel over 8 NeuronCores: each core gets 8192 rows of x. The
host casts x to bf16 before upload and upcasts the bf16 result back to
f32 after the gather; the device reads bf16, row-sum-reduces in f32 on
the Vector engine, broadcasts across the feature dim (split between the
Vector and Scalar engines), and stores bf16.

Precision: bf16 input adds ~2e-3 l2 to the row sums (independent
rounding over 1024 summands) and bf16 output ~1.7e-3; total measured
l2 rel err 2.4e-3 vs the 2e-2 tolerance (8x margin).

Perf notes (HW-measured):
- bf16 on BOTH sides cuts per-core HBM traffic to 32 MiB (vs 64 f32):
  16 MiB read + 16 MiB write. Exec is ~102-108 us in all device states
  (vs 132 us uncontended / 150 us sibling-contended for the f32-read
  version) -- the kernel is now compute/DMA balanced, so the sibling
  NeuronCore's share of the HBM stack no longer binds.
- The broadcast-cast is split 320/704 columns between Vector and Scalar
  so neither engine exceeds the ~80 us data window (DVE also carries
  the 68 us of 1x-mode reduces).
- Loads: [128, 4, 1024] bf16 tiles on the SP HWDGE ring; stores grouped
  two tiles per DMA ([128, 8, 1024] -> 16 KiB per-partition
  descriptors) on the ACT ring.
- The last two tiles store individually and the final tile's
  reduce/cast/store chain is split in halves (first half's cast on the
  Scalar engine) to minimize the end-of-stream serial tail.

Layout: partition p owns 64 consecutive DRAM rows (rearrange
"(p n) d -> p n d"), so each DMA moves contiguous bytes per partition.
"""

import ml_dtypes
import numpy as np

import concourse.tile as tile
from concourse import bacc, mybir
from concourse.bass_utils import run_bass_kernel_spmd

N_CORES = 8
BATCH = 65536
FEAT = 1024
ROWS = BATCH // N_CORES        # 8192 rows per core
P = 128                        # SBUF partitions
ROWS_PER_PART = ROWS // P      # 64 consecutive rows owned by each partition

R = 4                          # rows-per-partition per load tile
N_ITER = ROWS_PER_PART // R    # 16 load tiles
IN_BUFS = 6
OUT_BUFS = 3
VEC_COLS = 320                 # broadcast-cast columns on Vector; rest on Scalar

_nc_cache = []


def _build():
    nc = bacc.Bacc()
    x = nc.declare_dram_parameter("x", [ROWS, FEAT], mybir.dt.bfloat16, isOutput=False)
    y = nc.declare_dram_parameter("y", [ROWS, FEAT], mybir.dt.bfloat16, isOutput=True)
    xv = x[:, :].rearrange("(p n) d -> p n d", p=P)
    yv = y[:, :].rearrange("(p n) d -> p n d", p=P)

    # Pairs of load tiles share one store DMA (16 KiB bf16 descriptors);
    # the last two tiles store individually to keep the tail chain short.
    groups = [[i, i + 1] for i in range(0, N_ITER - 2, 2)] + [[N_ITER - 2], [N_ITER - 1]]

    def bcast(s_ap, o, a, b, rows, tail_first_half):
        src_v = s_ap.to_broadcast([P, rows, VEC_COLS])
        src_s = s_ap.to_broadcast([P, rows, FEAT - VEC_COLS])
        if tail_first_half:
            nc.scalar.copy(o[:, a:b, :], s_ap.to_broadcast([P, rows, FEAT]))
        else:
            nc.vector.tensor_copy(out=o[:, a:b, :VEC_COLS], in_=src_v)
            nc.scalar.copy(o[:, a:b, VEC_COLS:], src_s)

    with tile.TileContext(nc) as tc:
        with (
            tc.tile_pool(name="inp", bufs=IN_BUFS) as inp,
            tc.tile_pool(name="outp", bufs=OUT_BUFS) as outp,
            tc.tile_pool(name="sums", bufs=4) as sums_pool,
        ):
            for grp in groups:
                gr = R * len(grp)
                o = outp.tile([P, gr, FEAT], mybir.dt.bfloat16, name="o", tag="o")
                for gi, it in enumerate(grp):
                    t = inp.tile([P, R, FEAT], mybir.dt.bfloat16, name="t", tag="t")
                    nc.sync.dma_start(
                        out=t[:, :, :], in_=xv[:, it * R : (it + 1) * R, :]
                    )

                    s = sums_pool.tile([P, R], mybir.dt.float32, name="s", tag="s")
                    last = it == N_ITER - 1
                    k = 2 if last else 1
                    step = R // k
                    for j in range(k):
                        la, lb = j * step, (j + 1) * step          # within t/s
                        a, b = gi * R + la, gi * R + lb            # within o
                        nc.vector.reduce_sum(
                            out=s[:, la:lb],
                            in_=t[:, la:lb, :],
                            axis=mybir.AxisListType.X,
                        )
                        bcast(s[:, la:lb], o, a, b, step, k > 1 and j == 0)
                        if k > 1:
                            nc.scalar.dma_start(
                                out=yv[:, it * R + la : it * R + lb, :],
                                in_=o[:, a:b, :],
                            )
                    if k == 1 and gi == len(grp) - 1:
                        base = grp[0] * R
                        nc.scalar.dma_start(
                            out=yv[:, base : base + gr, :], in_=o[:, :, :]
                        )
    nc.finalize()
    return nc


def _get_nc():
    if not _nc_cache:
        _nc_cache.append(_build())
    return _nc_cache[0]


def kernel(x: np.ndarray) -> np.ndarray:
    nc = _get_nc()
    x = np.asarray(x).astype(ml_dtypes.bfloat16)
    shards = np.split(np.ascontiguousarray(x), N_CORES, axis=0)
    in_maps = [{"x": s} for s in shards]
    res = run_bass_kernel_spmd(nc, in_maps, list(range(N_CORES)))
    out = np.concatenate([res.results[i]["y"] for i in range(N_CORES)], axis=0)
    return out.astype(np.float32)


# revision 10
# speedup vs baseline: 1.4409x; 1.0086x over previous
"""KAN layer (identity edges) Trainium2 kernel.

output[b, o] = sum_i x[b, i]  for all o  -- row-sum broadcast to (B, 1024).

Data-parallel over 8 NeuronCores: each core gets 8192 rows of x. The
host casts x to bf16 before upload and upcasts the bf16 result back to
f32 after the gather; the device reads bf16, row-sum-reduces in f32 on
the Vector engine, broadcasts across the feature dim (split between the
Vector and Scalar engines), and stores bf16.

Precision: bf16 input adds ~2e-3 l2 to the row sums (independent
rounding over 1024 summands) and bf16 output ~1.7e-3; total measured
l2 rel err 2.4e-3 vs the 2e-2 tolerance (8x margin).

Perf notes (HW-measured):
- bf16 on BOTH sides cuts per-core HBM traffic to 32 MiB (vs 64 f32):
  16 MiB read + 16 MiB write. Exec is ~102-108 us in all device states
  (vs 132 us uncontended / 150 us sibling-contended for the f32-read
  version) -- the kernel is now compute/DMA balanced, so the sibling
  NeuronCore's share of the HBM stack no longer binds.
- The broadcast-cast is split 320/704 columns between Vector and Scalar
  so neither engine exceeds the ~80 us data window (DVE also carries
  the 68 us of 1x-mode reduces).
- Loads: [128, 4, 1024] bf16 tiles on the SP HWDGE ring; stores grouped
  two tiles per DMA ([128, 8, 1024] -> 16 KiB per-partition
  descriptors) on the ACT ring.
- The last two tiles store individually and the final tile's
  reduce/cast/store chain is split in halves (first half's cast on the
  Scalar engine) to minimize the end-of-stream serial tail.

Layout: partition p owns 64 consecutive DRAM rows (rearrange
"(p n) d -> p n d"), so each DMA moves contiguous bytes per partition.
"""

import ml_dtypes
import numpy as np

import concourse.tile as tile
from concourse import bacc, mybir
from concourse.bass_utils import run_bass_kernel_spmd

N_CORES = 8
BATCH = 65536
FEAT = 1024
ROWS = BATCH // N_CORES        # 8192 rows per core
P = 128                        # SBUF partitions
ROWS_PER_PART = ROWS // P      # 64 consecutive rows owned by each partition

R = 4                          # rows-per-partition per load tile
N_ITER = ROWS_PER_PART // R    # 16 load tiles
IN_BUFS = 8
OUT_BUFS = 4

_nc_cache = []


def _build():
    nc = bacc.Bacc()
    x = nc.declare_dram_parameter("x", [ROWS, FEAT], mybir.dt.bfloat16, isOutput=False)
    y = nc.declare_dram_parameter("y", [ROWS, FEAT], mybir.dt.bfloat16, isOutput=True)
    xv = x[:, :].rearrange("(p n) d -> p n d", p=P)
    yv = y[:, :].rearrange("(p n) d -> p n d", p=P)

    # Pairs of load tiles share one store DMA (16 KiB bf16 descriptors);
    # the last two tiles store individually to keep the tail chain short.
    groups = [[i, i + 1] for i in range(0, N_ITER - 2, 2)] + [[N_ITER - 2], [N_ITER - 1]]

    def bcast(s_ap, o, a, b, rows, tail_first_half):
        # All broadcast-casts on the Scalar (ACT) engine: measured ~269
        # G elem/s, and this leaves the Vector engine with only the 1x
        # reduces -- both engines ~70 us, under the ~80-94 us DMA window.
        nc.scalar.copy(o[:, a:b, :], s_ap.to_broadcast([P, rows, FEAT]))

    with tile.TileContext(nc) as tc:
        with (
            tc.tile_pool(name="inp", bufs=IN_BUFS) as inp,
            tc.tile_pool(name="outp", bufs=OUT_BUFS) as outp,
            tc.tile_pool(name="sums", bufs=4) as sums_pool,
        ):
            for grp in groups:
                gr = R * len(grp)
                o = outp.tile([P, gr, FEAT], mybir.dt.bfloat16, name="o", tag="o")
                for gi, it in enumerate(grp):
                    t = inp.tile([P, R, FEAT], mybir.dt.bfloat16, name="t", tag="t")
                    nc.sync.dma_start(
                        out=t[:, :, :], in_=xv[:, it * R : (it + 1) * R, :]
                    )

                    s = sums_pool.tile([P, R], mybir.dt.float32, name="s", tag="s")
                    last = it == N_ITER - 1
                    k = 2 if last else 1
                    step = R // k
                    for j in range(k):
                        la, lb = j * step, (j + 1) * step          # within t/s
                        a, b = gi * R + la, gi * R + lb            # within o
                        nc.vector.reduce_sum(
                            out=s[:, la:lb],
                            in_=t[:, la:lb, :],
                            axis=mybir.AxisListType.X,
                        )
                        bcast(s[:, la:lb], o, a, b, step, k > 1 and j == 0)
                        if k > 1:
                            nc.scalar.dma_start(
                                out=yv[:, it * R + la : it * R + lb, :],
                                in_=o[:, a:b, :],
                            )
                    if k == 1 and gi == len(grp) - 1:
                        base = grp[0] * R
                        nc.scalar.dma_start(
                            out=yv[:, base : base + gr, :], in_=o[:, :, :]
                        )
    nc.finalize()
    return nc


def _get_nc():
    if not _nc_cache:
        _nc_cache.append(_build())
    return _nc_cache[0]


def kernel(x: np.ndarray) -> np.ndarray:
    nc = _get_nc()
    x = np.asarray(x).astype(ml_dtypes.bfloat16)
    shards = np.split(np.ascontiguousarray(x), N_CORES, axis=0)
    in_maps = [{"x": s} for s in shards]
    res = run_bass_kernel_spmd(nc, in_maps, list(range(N_CORES)))
    out = np.concatenate([res.results[i]["y"] for i in range(N_CORES)], axis=0)
    return out.astype(np.float32)
